# revision 15
# baseline (speedup 1.0000x reference)
"""Sliding-window causal attention (B=2,T=2048,C=1024,H=16,D=64,W=256) on 8 trn2 cores.

Sharding: core c = (batch b = c//4, head-group g = c%4 of 4 heads).
Each core computes q/k/v projections for its 4 heads on its batch, windowed
attention per head, and a partial output projection (its 256 channels of the
contraction); host sums the 4 partials per batch.

Layout strategy (no on-chip transposes):
  - host ships xT = x[b].T  [C, T] bf16  (lhsT/rhs for projections)
  - qT, kT computed transposed [256, T] (channels on partitions)
  - v computed natural [T, 256] (tokens on partitions) -> PV lhsT
  - scores computed transposed S^T[k, q] so softmax sum comes from a
    ones-row folded into the PV matmul; per-q reciprocal broadcast along
    partitions via gpsimd.partition_broadcast.
"""

import os
import sys

sys.path.insert(0, "/opt/trn_rl_repo")

import numpy as np
import ml_dtypes

import concourse.bass as bass
import concourse.tile as tile
from concourse import bacc
from concourse import mybir
from concourse.bass import ds, ts

BF16 = ml_dtypes.bfloat16

B, T, C = 2, 2048, 1024
H, W, D = 16, 256, 64
HPC = 4          # heads per core
CL = HPC * D     # 256 local channels per core
NKT = C // 128   # 8 contraction tiles for projections
NT = T // 128    # 16 token tiles
SCALE = 0.125    # 1/sqrt(D)
F32 = mybir.dt.float32
BF = mybir.dt.bfloat16
USE_GPSIMD_BCAST = os.environ.get("ATTN_GPSIMD_BCAST", "0") == "1"


def build_program():
    nc = bacc.Bacc("TRN2", target_bir_lowering=False, debug=False)

    xT_d = nc.dram_tensor("xT", [C, T], BF, kind="ExternalInput")
    wqT_d = nc.dram_tensor("wqT", [C, CL], BF, kind="ExternalInput")
    wkT_d = nc.dram_tensor("wkT", [C, CL], BF, kind="ExternalInput")
    wvT_d = nc.dram_tensor("wvT", [C, CL], BF, kind="ExternalInput")
    woT_d = nc.dram_tensor("woT", [CL, C], BF, kind="ExternalInput")
    maskd_d = nc.dram_tensor("maskd2", [128, 256], BF, kind="ExternalInput")
    masks_d = nc.dram_tensor("masks2", [128, 256], BF, kind="ExternalInput")
    y_d = nc.dram_tensor("y", [T, C], F32, kind="ExternalOutput")

    with tile.TileContext(nc) as tc:
        with (
            tc.tile_pool(name="const", bufs=1) as constp,
            tc.tile_pool(name="acts", bufs=1) as actsp,
            tc.tile_pool(name="epool", bufs=8) as ep,
            tc.tile_pool(name="small", bufs=4) as smallp,
            tc.tile_pool(name="ysb", bufs=3) as yp,
            tc.tile_pool(name="psA", bufs=2, space="PSUM") as psA,
            tc.tile_pool(name="psB", bufs=4, space="PSUM") as psB,
            tc.tile_pool(name="dramp", bufs=4, space="DRAM") as dramp,
        ):
            if USE_GPSIMD_BCAST:
                from concourse import library_config

                nc.gpsimd.load_library(library_config.attn)
            # ---- static SBUF tiles + loads ----
            # small weights/masks first so the first proj matmuls start early
            maskd_sb = constp.tile([128, 256], BF, tag="maskd", name="maskd_sb")
            masks_sb = constp.tile([128, 256], BF, tag="masks", name="masks_sb")
            nc.sync.dma_start(maskd_sb[:], maskd_d[:])
            nc.sync.dma_start(masks_sb[:], masks_d[:])
            wq_sb, wk_sb, wv_sb = [], [], []
            for name, dram, lst in (
                ("wq", wqT_d, wq_sb),
                ("wk", wkT_d, wk_sb),
                ("wv", wvT_d, wv_sb),
            ):
                for i in range(NKT):
                    t_ = constp.tile([128, CL], BF, tag=f"{name}{i}", name=f"{name}{i}")
                    nc.sync.dma_start(t_[:], dram[ts(i, 128), :])
                    lst.append(t_)
            wo_sb = []
            for j in range(2):
                t_ = constp.tile([128, C], BF, tag=f"wo{j}", name=f"wo{j}")
                nc.sync.dma_start(t_[:], woT_d[ts(j, 128), :])
                wo_sb.append(t_)
            xT_sb = []
            for i in range(NKT):
                t_ = constp.tile([128, T], BF, tag=f"xT{i}", name=f"xT{i}")
                xT_sb.append(t_)
            # column-chunked loads ordered to unblock proj groups asap
            for n in range(4):
                for i in range(NKT):
                    nc.sync.dma_start(
                        xT_sb[i][:, ts(n, 512)], xT_d[ts(i, 128), ts(n, 512)]
                    )
            maskd_v = maskd_sb.rearrange("p (b x) -> p b x", b=2)
            masks_v = masks_sb.rearrange("p (b x) -> p b x", b=2)
            ones_sb = constp.tile([1, 64], F32, tag="ones", name="ones_sb")
            nc.gpsimd.memset(ones_sb[:], 1.0)

            # persistent activations
            qT_sb = [actsp.tile([128, T], BF, tag=f"qT{m}", name=f"qT{m}") for m in range(2)]
            kT_sb = [actsp.tile([128, T], BF, tag=f"kT{m}", name=f"kT{m}") for m in range(2)]
            aT_sb = [actsp.tile([128, T], BF, tag=f"aT{m}", name=f"aT{m}") for m in range(2)]
            # v natural layout, per token-tile: [v_h0(64)|1|v_h1(64)|1|...] = 260 cols
            v_sb = [actsp.tile([128, 4 * 65], BF, tag=f"v{t}", name=f"v{t}") for t in range(NT)]
            for t in range(NT):
                vv = v_sb[t].rearrange("p (h c) -> p h c", h=4)
                nc.gpsimd.memset(vv[:, :, 64:65], 1.0)

            # ---- phase 1: projections ----
            # qT/kT: [256, T] transposed; out m-tile rows = 2 heads
            for w_sb, dstT in ((wq_sb, qT_sb), (wk_sb, kT_sb)):
                for m in range(2):
                    for n in range(4):
                        ps = psA.tile([128, 512], F32, tag="psA", name="ps_proj")
                        for kt in range(NKT):
                            nc.tensor.matmul(
                                ps[:],
                                lhsT=w_sb[kt][:, ts(m, 128)],
                                rhs=xT_sb[kt][:, ts(n, 512)],
                                start=(kt == 0),
                                stop=(kt == NKT - 1),
                            )
                        nc.scalar.copy(dstT[m][:, ts(n, 512)], ps[:])
            # v natural: [T, 256]
            for t in range(NT):
                ps = psB.tile([128, CL], F32, tag="pv", name="ps_v")
                for kt in range(NKT):
                    nc.tensor.matmul(
                        ps[:],
                        lhsT=xT_sb[kt][:, ts(t, 128)],
                        rhs=wv_sb[kt][:],
                        start=(kt == 0),
                        stop=(kt == NKT - 1),
                    )
                vv = v_sb[t].rearrange("p (h c) -> p h c", h=4)
                nc.vector.tensor_copy(
                    vv[:, :, 0:64], ps.rearrange("p (h c) -> p h c", h=4)[:]
                )

            # ---- phase 2: attention per head pair ----
            QB = 512  # norm/evac granularity along q (4 query tiles)

            def pv_step(h, j, e_tiles, pvps):
                """PV accumulation for head h (0..3), query tile j."""
                hp = h % 2
                if j % 4 == 0:
                    pvps[h] = psB.tile([65, QB], F32, tag="pv", name="ps_pv")
                ps = pvps[h]
                col = 128 * (j % 4)
                kts = [k2 for k2 in (j - 2, j - 1, j) if k2 >= 0]
                for idx, k2 in enumerate(kts):
                    nc.tensor.matmul(
                        ps[:, ds(col, 128)],
                        lhsT=v_sb[k2][:, ds(65 * h, 65)],
                        rhs=e_tiles[k2][:, ds(384 * hp + 128 * (j - k2), 128)],
                        start=(idx == 0),
                        stop=(idx == len(kts) - 1),
                    )
                if j % 4 == 3:
                    blk = j // 4
                    # custom-DVE recip is wrong from PSUM; stage row via ACT
                    d_sb = smallp.tile([1, QB], F32, tag="d", name="d_row")
                    nc.scalar.copy(d_sb[:], ps[64:65, :])
                    r = smallp.tile([1, QB], F32, tag="r", name="r_row")
                    nc.vector.reciprocal_approx_fast(r[:], d_sb[:])
                    # partition-broadcast via DRAM bounce (SBUF APs can't step-0)
                    r_dr = dramp.tile([1, QB], F32, tag="rdr", name="r_dr")
                    nc.sync.dma_start(r_dr[:], r[:])
                    rb = smallp.tile([64, QB], F32, tag="rb", name="rb_bc")
                    nc.sync.dma_start(rb[:], r_dr[:].to_broadcast([64, QB]))
                    row = 64 * (h % 2)
                    nc.vector.tensor_mul(
                        aT_sb[h // 2][ds(row, 64), ds(QB * blk, QB)],
                        ps[0:64, :],
                        rb[:],
                    )

            for mp in range(2):
                hA, hB = 2 * mp, 2 * mp + 1
                e_tiles = []
                pvps = {}
                for kt in range(NT):
                    nkt = 128 * min(3, NT - kt)
                    sc = psA.tile([128, 1024], F32, tag="psA", name="ps_sc")
                    for half in range(2):
                        rows = slice(64 * half, 64 * half + 64)
                        nc.tensor.matmul(
                            sc[:, ds(512 * half, nkt)],
                            lhsT=kT_sb[mp][rows, ts(kt, 128)],
                            rhs=qT_sb[mp][rows, ds(128 * kt, nkt)],
                            start=True,
                            stop=True,
                        )
                    E = ep.tile([128, 768], BF, tag="E", name="E")
                    scv = sc.rearrange("p (b x) -> p b x", b=2)
                    Ev = E.rearrange("p (b x) -> p b x", b=2)
                    nc.scalar.activation(
                        Ev[:, :, 0:nkt],
                        scv[:, :, 0:nkt],
                        mybir.ActivationFunctionType.Exp,
                        scale=SCALE,
                    )
                    # mask diag subtile (q-tile kt), always
                    nc.vector.tensor_mul(
                        Ev[:, :, 0:128], Ev[:, :, 0:128], maskd_v[:]
                    )
                    # mask strict subtile (q-tile kt+2) when it exists
                    if kt <= NT - 3:
                        nc.vector.tensor_mul(
                            Ev[:, :, 256:384], Ev[:, :, 256:384], masks_v[:]
                        )
                    e_tiles.append(E)
                    pv_step(hA, kt, e_tiles, pvps)
                    pv_step(hB, kt, e_tiles, pvps)

            # ---- phase 3: output projection (partial y) ----
            for t in range(NT):
                ysb = yp.tile([128, C], F32, tag="y", name="ysb")
                for n in range(2):
                    ps = psB.tile([128, 512], F32, tag="pv", name="ps_y")
                    for kj in range(2):
                        nc.tensor.matmul(
                            ps[:],
                            lhsT=aT_sb[kj][:, ts(t, 128)],
                            rhs=wo_sb[kj][:, ts(n, 512)],
                            start=(kj == 0),
                            stop=(kj == 1),
                        )
                    if (2 * t + n) % 2 == 0:
                        nc.scalar.copy(ysb[:, ts(n, 512)], ps[:])
                    else:
                        nc.vector.tensor_copy(ysb[:, ts(n, 512)], ps[:])
                nc.sync.dma_start(y_d[ts(t, 128), :], ysb[:])

    nc.compile()
    return nc


def make_masks():
    one = np.ones((128, 128), np.float32)
    maskd = np.triu(one)          # keep iff i >= kk  (diag tile)
    masks_ = np.tril(one, -1)     # keep iff i <  kk  (strict tile)
    md2 = np.concatenate([maskd, maskd], axis=1).astype(BF16)
    ms2 = np.concatenate([masks_, masks_], axis=1).astype(BF16)
    return md2, ms2


def make_in_maps(x, wq, wk, wv, wo):
    x = np.asarray(x, np.float32)
    wq, wk, wv, wo = (np.asarray(a, np.float32) for a in (wq, wk, wv, wo))
    md2, ms2 = make_masks()
    in_maps = []
    for c in range(8):
        b, g = divmod(c, 4)
        sl = slice(g * CL, (g + 1) * CL)
        in_maps.append(
            {
                "xT": np.ascontiguousarray(x[b].T).astype(BF16),
                "wqT": np.ascontiguousarray(wq[sl, :].T).astype(BF16),
                "wkT": np.ascontiguousarray(wk[sl, :].T).astype(BF16),
                "wvT": np.ascontiguousarray(wv[sl, :].T).astype(BF16),
                "woT": np.ascontiguousarray(wo[:, sl].T).astype(BF16),
                "maskd2": md2,
                "masks2": ms2,
            }
        )
    return in_maps


_PROG = None


def _get_prog():
    global _PROG
    if _PROG is None:
        _PROG = build_program()
    return _PROG


def kernel(x, wq, wk, wv, wo, _trace=False, _tmpdir=None):
    from concourse.bass_utils import run_bass_kernel_spmd

    nc = _get_prog()
    in_maps = make_in_maps(x, wq, wk, wv, wo)
    res = run_bass_kernel_spmd(
        nc, in_maps, core_ids=list(range(8)), trace=_trace, tmpdir=_tmpdir
    )
    y = np.zeros((B, T, C), np.float32)
    for c in range(8):
        b = c // 4
        y[b] += res.results[c]["y"]
    if _trace:
        kernel._last_results = res
    return y


# revision 16
# speedup vs baseline: 1.0749x; 1.0749x over previous
"""Sliding-window causal attention (B=2,T=2048,C=1024,H=16,D=64,W=256) on 8 trn2 cores.

Sharding: core c = (batch b = c//4, head-group g = c%4 of 4 heads).
Each core computes q/k/v projections for its 4 heads on its batch, windowed
attention per head, and a partial output projection (its 256 channels of the
contraction); host sums the 4 partials per batch.

Layout strategy (no on-chip transposes):
  - host ships xT = x[b].T  [C, T] bf16  (lhsT/rhs for projections)
  - qT, kT computed transposed [256, T] (channels on partitions)
  - v computed natural [T, 256] (tokens on partitions) -> PV lhsT
  - scores computed transposed S^T[k, q] so softmax sum comes from a
    ones-row folded into the PV matmul; per-q reciprocal broadcast along
    partitions via gpsimd.partition_broadcast.
"""

import os
import sys

sys.path.insert(0, "/opt/trn_rl_repo")

import numpy as np
import ml_dtypes

import concourse.bass as bass
import concourse.tile as tile
from concourse import bacc
from concourse import mybir
from concourse.bass import ds, ts

BF16 = ml_dtypes.bfloat16

B, T, C = 2, 2048, 1024
H, W, D = 16, 256, 64
HPC = 4          # heads per core
CL = HPC * D     # 256 local channels per core
NKT = C // 128   # 8 contraction tiles for projections
NT = T // 128    # 16 token tiles
SCALE = 0.125    # 1/sqrt(D)
F32 = mybir.dt.float32
BF = mybir.dt.bfloat16
USE_GPSIMD_BCAST = os.environ.get("ATTN_GPSIMD_BCAST", "0") == "1"


def build_program():
    nc = bacc.Bacc("TRN2", target_bir_lowering=False, debug=False)

    # consolidated inputs: SBUF-row-contiguous so DMA descriptors are 4-32KB
    xT_d = nc.dram_tensor("xTt", [128, NKT * T], BF, kind="ExternalInput")
    w_d = nc.dram_tensor("wt", [128, 3 * NKT * CL], BF, kind="ExternalInput")
    wo_d = nc.dram_tensor("wot", [128, 2 * C], BF, kind="ExternalInput")
    mask_d = nc.dram_tensor("maskt", [128, 512], BF, kind="ExternalInput")
    y_d = nc.dram_tensor("y", [T, C], F32, kind="ExternalOutput")

    with tile.TileContext(nc) as tc:
        with (
            tc.tile_pool(name="const", bufs=1) as constp,
            tc.tile_pool(name="acts", bufs=1) as actsp,
            tc.tile_pool(name="epool", bufs=8) as ep,
            tc.tile_pool(name="small", bufs=4) as smallp,
            tc.tile_pool(name="ysb", bufs=3) as yp,
            tc.tile_pool(name="psA", bufs=2, space="PSUM") as psA,
            tc.tile_pool(name="psB", bufs=4, space="PSUM") as psB,
            tc.tile_pool(name="dramp", bufs=4, space="DRAM") as dramp,
        ):
            if USE_GPSIMD_BCAST:
                from concourse import library_config

                nc.gpsimd.load_library(library_config.attn)
            # ---- static SBUF tiles + loads (few big DMAs, 4-32KB descriptors) ----
            mask_all = constp.tile([128, 512], BF, tag="maskall", name="mask_all")
            nc.sync.dma_start(mask_all[:], mask_d[:])
            w_all = constp.tile([128, 3 * NKT * CL], BF, tag="wall", name="w_all")
            nc.sync.dma_start(w_all[:], w_d[:])
            wo_all = constp.tile([128, 2 * C], BF, tag="woall", name="wo_all")
            nc.sync.dma_start(wo_all[:], wo_d[:])
            xT_all = constp.tile([128, NKT * T], BF, tag="xTall", name="xT_all")
            # two halves so the first proj groups unblock at ~half the load
            nc.sync.dma_start(xT_all[:, 0 : 4 * T], xT_d[:, 0 : 4 * T])
            nc.sync.dma_start(xT_all[:, 4 * T : 8 * T], xT_d[:, 4 * T : 8 * T])
            xT_sb = [xT_all[:, ds(i * T, T)] for i in range(NKT)]
            wq_sb = [w_all[:, ds((0 * NKT + i) * CL, CL)] for i in range(NKT)]
            wk_sb = [w_all[:, ds((1 * NKT + i) * CL, CL)] for i in range(NKT)]
            wv_sb = [w_all[:, ds((2 * NKT + i) * CL, CL)] for i in range(NKT)]
            wo_sb = [wo_all[:, ds(j * C, C)] for j in range(2)]
            maskd_sb = mask_all[:, 0:256]
            masks_sb = mask_all[:, 256:512]
            maskd_v = maskd_sb.rearrange("p (b x) -> p b x", b=2)
            masks_v = masks_sb.rearrange("p (b x) -> p b x", b=2)
            ones_sb = constp.tile([1, 64], F32, tag="ones", name="ones_sb")
            nc.gpsimd.memset(ones_sb[:], 1.0)

            # persistent activations
            qT_sb = [actsp.tile([128, T], BF, tag=f"qT{m}", name=f"qT{m}") for m in range(2)]
            kT_sb = [actsp.tile([128, T], BF, tag=f"kT{m}", name=f"kT{m}") for m in range(2)]
            aT_sb = [actsp.tile([128, T], BF, tag=f"aT{m}", name=f"aT{m}") for m in range(2)]
            # v natural layout, per token-tile: [v_h0(64)|1|v_h1(64)|1|...] = 260 cols
            v_sb = [actsp.tile([128, 4 * 65], BF, tag=f"v{t}", name=f"v{t}") for t in range(NT)]
            for t in range(NT):
                vv = v_sb[t].rearrange("p (h c) -> p h c", h=4)
                nc.gpsimd.memset(vv[:, :, 64:65], 1.0)

            # ---- phase 1: projections ----
            # qT/kT: [256, T] transposed; out m-tile rows = 2 heads
            for w_sb, dstT in ((wq_sb, qT_sb), (wk_sb, kT_sb)):
                for m in range(2):
                    for n in range(4):
                        ps = psA.tile([128, 512], F32, tag="psA", name="ps_proj")
                        for kt in range(NKT):
                            nc.tensor.matmul(
                                ps[:],
                                lhsT=w_sb[kt][:, ts(m, 128)],
                                rhs=xT_sb[kt][:, ts(n, 512)],
                                start=(kt == 0),
                                stop=(kt == NKT - 1),
                            )
                        nc.scalar.copy(dstT[m][:, ts(n, 512)], ps[:])
            # v natural: [T, 256]
            for t in range(NT):
                ps = psB.tile([128, CL], F32, tag="pv", name="ps_v")
                for kt in range(NKT):
                    nc.tensor.matmul(
                        ps[:],
                        lhsT=xT_sb[kt][:, ts(t, 128)],
                        rhs=wv_sb[kt][:],
                        start=(kt == 0),
                        stop=(kt == NKT - 1),
                    )
                vv = v_sb[t].rearrange("p (h c) -> p h c", h=4)
                nc.vector.tensor_copy(
                    vv[:, :, 0:64], ps.rearrange("p (h c) -> p h c", h=4)[:]
                )

            # ---- phase 2: attention per head pair ----
            QB = 512  # norm/evac granularity along q (4 query tiles)

            def pv_step(h, j, e_tiles, pvps):
                """PV accumulation for head h (0..3), query tile j."""
                hp = h % 2
                if j % 4 == 0:
                    pvps[h] = psB.tile([65, QB], F32, tag="pv", name="ps_pv")
                ps = pvps[h]
                col = 128 * (j % 4)
                kts = [k2 for k2 in (j - 2, j - 1, j) if k2 >= 0]
                for idx, k2 in enumerate(kts):
                    nc.tensor.matmul(
                        ps[:, ds(col, 128)],
                        lhsT=v_sb[k2][:, ds(65 * h, 65)],
                        rhs=e_tiles[k2][:, ds(384 * hp + 128 * (j - k2), 128)],
                        start=(idx == 0),
                        stop=(idx == len(kts) - 1),
                    )
                if j % 4 == 3:
                    blk = j // 4
                    # custom-DVE recip is wrong from PSUM; stage row via ACT
                    d_sb = smallp.tile([1, QB], F32, tag="d", name="d_row")
                    nc.scalar.copy(d_sb[:], ps[64:65, :])
                    r = smallp.tile([1, QB], F32, tag="r", name="r_row")
                    nc.vector.reciprocal_approx_fast(r[:], d_sb[:])
                    # partition-broadcast via DRAM bounce (SBUF APs can't step-0)
                    r_dr = dramp.tile([1, QB], F32, tag="rdr", name="r_dr")
                    nc.gpsimd.dma_start(r_dr[:], r[:])
                    rb = smallp.tile([64, QB], F32, tag="rb", name="rb_bc")
                    nc.gpsimd.dma_start(rb[:], r_dr[:].to_broadcast([64, QB]))
                    row = 64 * (h % 2)
                    nc.vector.tensor_mul(
                        aT_sb[h // 2][ds(row, 64), ds(QB * blk, QB)],
                        ps[0:64, :],
                        rb[:],
                    )

            for mp in range(2):
                hA, hB = 2 * mp, 2 * mp + 1
                e_tiles = []
                pvps = {}
                for kt in range(NT):
                    nkt = 128 * min(3, NT - kt)
                    sc = psA.tile([128, 1024], F32, tag="psA", name="ps_sc")
                    for half in range(2):
                        rows = slice(64 * half, 64 * half + 64)
                        nc.tensor.matmul(
                            sc[:, ds(512 * half, nkt)],
                            lhsT=kT_sb[mp][rows, ts(kt, 128)],
                            rhs=qT_sb[mp][rows, ds(128 * kt, nkt)],
                            start=True,
                            stop=True,
                        )
                    E = ep.tile([128, 768], BF, tag="E", name="E")
                    scv = sc.rearrange("p (b x) -> p b x", b=2)
                    Ev = E.rearrange("p (b x) -> p b x", b=2)
                    nc.scalar.activation(
                        Ev[:, :, 0:nkt],
                        scv[:, :, 0:nkt],
                        mybir.ActivationFunctionType.Exp,
                        scale=SCALE,
                    )
                    # mask diag subtile (q-tile kt), always
                    nc.vector.tensor_mul(
                        Ev[:, :, 0:128], Ev[:, :, 0:128], maskd_v[:]
                    )
                    # mask strict subtile (q-tile kt+2) when it exists
                    if kt <= NT - 3:
                        nc.vector.tensor_mul(
                            Ev[:, :, 256:384], Ev[:, :, 256:384], masks_v[:]
                        )
                    e_tiles.append(E)
                    pv_step(hA, kt, e_tiles, pvps)
                    pv_step(hB, kt, e_tiles, pvps)

            # ---- phase 3: output projection (partial y) ----
            for t in range(NT):
                ysb = yp.tile([128, C], F32, tag="y", name="ysb")
                for n in range(2):
                    ps = psB.tile([128, 512], F32, tag="pv", name="ps_y")
                    for kj in range(2):
                        nc.tensor.matmul(
                            ps[:],
                            lhsT=aT_sb[kj][:, ts(t, 128)],
                            rhs=wo_sb[kj][:, ts(n, 512)],
                            start=(kj == 0),
                            stop=(kj == 1),
                        )
                    if (2 * t + n) % 2 == 0:
                        nc.scalar.copy(ysb[:, ts(n, 512)], ps[:])
                    else:
                        nc.vector.tensor_copy(ysb[:, ts(n, 512)], ps[:])
                nc.sync.dma_start(y_d[ts(t, 128), :], ysb[:])

    nc.compile()
    return nc


def make_masks():
    one = np.ones((128, 128), np.float32)
    maskd = np.triu(one)          # keep iff i >= kk  (diag tile)
    masks_ = np.tril(one, -1)     # keep iff i <  kk  (strict tile)
    md2 = np.concatenate([maskd, maskd], axis=1).astype(BF16)
    ms2 = np.concatenate([masks_, masks_], axis=1).astype(BF16)
    return md2, ms2


def make_in_maps(x, wq, wk, wv, wo):
    x = np.asarray(x, np.float32)
    wq, wk, wv, wo = (np.asarray(a, np.float32) for a in (wq, wk, wv, wo))
    md2, ms2 = make_masks()
    mask_all = np.hstack([md2, ms2])  # [128, 512]

    def tile_rows(a):  # [1024, W] -> [128, 8*W] (row-blocks side by side)
        return np.hstack([a[i * 128 : (i + 1) * 128] for i in range(a.shape[0] // 128)])

    in_maps = []
    for c in range(8):
        b, g = divmod(c, 4)
        sl = slice(g * CL, (g + 1) * CL)
        xTt = tile_rows(np.ascontiguousarray(x[b].T).astype(BF16))
        wt = np.hstack(
            [
                tile_rows(np.ascontiguousarray(w[sl, :].T).astype(BF16))
                for w in (wq, wk, wv)
            ]
        )
        wot = tile_rows(np.ascontiguousarray(wo[:, sl].T).astype(BF16))
        in_maps.append(
            {"xTt": xTt, "wt": wt, "wot": wot, "maskt": mask_all}
        )
    return in_maps


_PROG = None


def _get_prog():
    global _PROG
    if _PROG is None:
        _PROG = build_program()
    return _PROG


def kernel(x, wq, wk, wv, wo, _trace=False, _tmpdir=None):
    from concourse.bass_utils import run_bass_kernel_spmd

    nc = _get_prog()
    in_maps = make_in_maps(x, wq, wk, wv, wo)
    res = run_bass_kernel_spmd(
        nc, in_maps, core_ids=list(range(8)), trace=_trace, tmpdir=_tmpdir
    )
    y = np.zeros((B, T, C), np.float32)
    for c in range(8):
        b = c // 4
        y[b] += res.results[c]["y"]
    if _trace:
        kernel._last_results = res
    return y


# revision 17
# speedup vs baseline: 1.1221x; 1.0439x over previous
"""Sliding-window causal attention (B=2,T=2048,C=1024,H=16,D=64,W=256) on 8 trn2 cores.

Sharding: core c = (batch b = c//4, head-group g = c%4 of 4 heads).
Each core computes q/k/v projections for its 4 heads on its batch, windowed
attention per head, and a partial output projection (its 256 channels of the
contraction); host sums the 4 partials per batch.

Layout strategy (no on-chip transposes):
  - host ships xT = x[b].T  [C, T] bf16  (lhsT/rhs for projections)
  - qT, kT computed transposed [256, T] (channels on partitions)
  - v computed natural [T, 256] (tokens on partitions) -> PV lhsT
  - scores computed transposed S^T[k, q] so softmax sum comes from a
    ones-row folded into the PV matmul; per-q reciprocal broadcast along
    partitions via gpsimd.partition_broadcast.
"""

import os
import sys

sys.path.insert(0, "/opt/trn_rl_repo")

import numpy as np
import ml_dtypes

import concourse.bass as bass
import concourse.tile as tile
from concourse import bacc
from concourse import mybir
from concourse.bass import ds, ts

BF16 = ml_dtypes.bfloat16

B, T, C = 2, 2048, 1024
H, W, D = 16, 256, 64
HPC = 4          # heads per core
CL = HPC * D     # 256 local channels per core
NKT = C // 128   # 8 contraction tiles for projections
NT = T // 128    # 16 token tiles
SCALE = 0.125    # 1/sqrt(D)
F32 = mybir.dt.float32
BF = mybir.dt.bfloat16
USE_GPSIMD_BCAST = os.environ.get("ATTN_GPSIMD_BCAST", "0") == "1"


def build_program():
    nc = bacc.Bacc("TRN2", target_bir_lowering=False, debug=False)

    # consolidated inputs: SBUF-row-contiguous so DMA descriptors are 4-32KB
    xT_d = nc.dram_tensor("xTt", [128, NKT * T], BF, kind="ExternalInput")
    w_d = nc.dram_tensor("wt", [128, 3 * NKT * CL], BF, kind="ExternalInput")
    wo_d = nc.dram_tensor("wot", [128, 2 * C], BF, kind="ExternalInput")
    mask_d = nc.dram_tensor("maskt", [128, 512], BF, kind="ExternalInput")
    y_d = nc.dram_tensor("y", [T, C], BF, kind="ExternalOutput")

    with tile.TileContext(nc) as tc:
        with (
            tc.tile_pool(name="const", bufs=1) as constp,
            tc.tile_pool(name="acts", bufs=1) as actsp,
            tc.tile_pool(name="epool", bufs=8) as ep,
            tc.tile_pool(name="small", bufs=4) as smallp,
            tc.tile_pool(name="ysb", bufs=3) as yp,
            tc.tile_pool(name="psA", bufs=2, space="PSUM") as psA,
            tc.tile_pool(name="psB", bufs=4, space="PSUM") as psB,
            tc.tile_pool(name="dramp", bufs=4, space="DRAM") as dramp,
        ):
            if USE_GPSIMD_BCAST:
                from concourse import library_config

                nc.gpsimd.load_library(library_config.attn)
            # ---- static SBUF tiles + loads (few big DMAs, 4-32KB descriptors) ----
            mask_all = constp.tile([128, 512], BF, tag="maskall", name="mask_all")
            nc.sync.dma_start(mask_all[:], mask_d[:])
            w_all = constp.tile([128, 3 * NKT * CL], BF, tag="wall", name="w_all")
            nc.sync.dma_start(w_all[:], w_d[:])
            wo_all = constp.tile([128, 2 * C], BF, tag="woall", name="wo_all")
            nc.sync.dma_start(wo_all[:], wo_d[:])
            xT_all = constp.tile([128, NKT * T], BF, tag="xTall", name="xT_all")
            # two halves so the first proj groups unblock at ~half the load
            nc.sync.dma_start(xT_all[:, 0 : 4 * T], xT_d[:, 0 : 4 * T])
            nc.sync.dma_start(xT_all[:, 4 * T : 8 * T], xT_d[:, 4 * T : 8 * T])
            xT_sb = [xT_all[:, ds(i * T, T)] for i in range(NKT)]
            wq_sb = [w_all[:, ds((0 * NKT + i) * CL, CL)] for i in range(NKT)]
            wk_sb = [w_all[:, ds((1 * NKT + i) * CL, CL)] for i in range(NKT)]
            wv_sb = [w_all[:, ds((2 * NKT + i) * CL, CL)] for i in range(NKT)]
            wo_sb = [wo_all[:, ds(j * C, C)] for j in range(2)]
            maskd_sb = mask_all[:, 0:256]
            masks_sb = mask_all[:, 256:512]
            maskd_v = maskd_sb.rearrange("p (b x) -> p b x", b=2)
            masks_v = masks_sb.rearrange("p (b x) -> p b x", b=2)
            ones_sb = constp.tile([1, 64], F32, tag="ones", name="ones_sb")
            nc.gpsimd.memset(ones_sb[:], 1.0)

            # persistent activations
            qT_sb = [actsp.tile([128, T], BF, tag=f"qT{m}", name=f"qT{m}") for m in range(2)]
            kT_sb = [actsp.tile([128, T], BF, tag=f"kT{m}", name=f"kT{m}") for m in range(2)]
            aT_sb = [actsp.tile([128, T], BF, tag=f"aT{m}", name=f"aT{m}") for m in range(2)]
            # v natural layout, per token-tile: [v_h0(64)|1|v_h1(64)|1|...] = 260 cols
            v_sb = [actsp.tile([128, 4 * 65], BF, tag=f"v{t}", name=f"v{t}") for t in range(NT)]
            for t in range(NT):
                vv = v_sb[t].rearrange("p (h c) -> p h c", h=4)
                nc.gpsimd.memset(vv[:, :, 64:65], 1.0)

            # ---- phase 1: projections ----
            # qT/kT: [256, T] transposed; out m-tile rows = 2 heads
            for w_sb, dstT in ((wq_sb, qT_sb), (wk_sb, kT_sb)):
                for m in range(2):
                    for n in range(4):
                        ps = psA.tile([128, 512], F32, tag="psA", name="ps_proj")
                        for kt in range(NKT):
                            nc.tensor.matmul(
                                ps[:],
                                lhsT=w_sb[kt][:, ts(m, 128)],
                                rhs=xT_sb[kt][:, ts(n, 512)],
                                start=(kt == 0),
                                stop=(kt == NKT - 1),
                            )
                        nc.scalar.copy(dstT[m][:, ts(n, 512)], ps[:])
            # v natural: [T, 256]
            for t in range(NT):
                ps = psB.tile([128, CL], F32, tag="pv", name="ps_v")
                for kt in range(NKT):
                    nc.tensor.matmul(
                        ps[:],
                        lhsT=xT_sb[kt][:, ts(t, 128)],
                        rhs=wv_sb[kt][:],
                        start=(kt == 0),
                        stop=(kt == NKT - 1),
                    )
                vv = v_sb[t].rearrange("p (h c) -> p h c", h=4)
                nc.vector.tensor_copy(
                    vv[:, :, 0:64], ps.rearrange("p (h c) -> p h c", h=4)[:]
                )

            # ---- phase 2: attention per head pair ----
            QB = 512  # norm/evac granularity along q (4 query tiles)

            def pv_step(h, j, e_tiles, pvps):
                """PV accumulation for head h (0..3), query tile j."""
                hp = h % 2
                if j % 4 == 0:
                    pvps[h] = psB.tile([65, QB], F32, tag="pv", name="ps_pv")
                ps = pvps[h]
                col = 128 * (j % 4)
                kts = [k2 for k2 in (j - 2, j - 1, j) if k2 >= 0]
                for idx, k2 in enumerate(kts):
                    nc.tensor.matmul(
                        ps[:, ds(col, 128)],
                        lhsT=v_sb[k2][:, ds(65 * h, 65)],
                        rhs=e_tiles[k2][:, ds(384 * hp + 128 * (j - k2), 128)],
                        start=(idx == 0),
                        stop=(idx == len(kts) - 1),
                    )
                if j % 4 == 3:
                    blk = j // 4
                    # custom-DVE recip is wrong from PSUM; stage row via ACT
                    d_sb = smallp.tile([1, QB], F32, tag="d", name="d_row")
                    nc.scalar.copy(d_sb[:], ps[64:65, :])
                    r = smallp.tile([1, QB], F32, tag="r", name="r_row")
                    nc.vector.reciprocal_approx_fast(r[:], d_sb[:])
                    # partition-broadcast via DRAM bounce (SBUF APs can't step-0)
                    r_dr = dramp.tile([1, QB], F32, tag="rdr", name="r_dr")
                    nc.gpsimd.dma_start(r_dr[:], r[:])
                    rb = smallp.tile([64, QB], F32, tag="rb", name="rb_bc")
                    nc.gpsimd.dma_start(rb[:], r_dr[:].to_broadcast([64, QB]))
                    row = 64 * (h % 2)
                    nc.vector.tensor_mul(
                        aT_sb[h // 2][ds(row, 64), ds(QB * blk, QB)],
                        ps[0:64, :],
                        rb[:],
                    )

            for mp in range(2):
                hA, hB = 2 * mp, 2 * mp + 1
                e_tiles = []
                pvps = {}
                for kt in range(NT):
                    nkt = 128 * min(3, NT - kt)
                    sc = psA.tile([128, 1024], F32, tag="psA", name="ps_sc")
                    for half in range(2):
                        rows = slice(64 * half, 64 * half + 64)
                        nc.tensor.matmul(
                            sc[:, ds(512 * half, nkt)],
                            lhsT=kT_sb[mp][rows, ts(kt, 128)],
                            rhs=qT_sb[mp][rows, ds(128 * kt, nkt)],
                            start=True,
                            stop=True,
                        )
                    E = ep.tile([128, 768], BF, tag="E", name="E")
                    scv = sc.rearrange("p (b x) -> p b x", b=2)
                    Ev = E.rearrange("p (b x) -> p b x", b=2)
                    nc.scalar.activation(
                        Ev[:, :, 0:nkt],
                        scv[:, :, 0:nkt],
                        mybir.ActivationFunctionType.Exp,
                        scale=SCALE,
                    )
                    # mask diag subtile (q-tile kt), always
                    nc.vector.tensor_mul(
                        Ev[:, :, 0:128], Ev[:, :, 0:128], maskd_v[:]
                    )
                    # mask strict subtile (q-tile kt+2) when it exists
                    if kt <= NT - 3:
                        nc.vector.tensor_mul(
                            Ev[:, :, 256:384], Ev[:, :, 256:384], masks_v[:]
                        )
                    e_tiles.append(E)
                    pv_step(hA, kt, e_tiles, pvps)
                    pv_step(hB, kt, e_tiles, pvps)

            # ---- phase 3: output projection (partial y) ----
            for t in range(NT):
                ysb = yp.tile([128, C], BF, tag="y", name="ysb")
                for n in range(2):
                    ps = psB.tile([128, 512], F32, tag="pv", name="ps_y")
                    for kj in range(2):
                        nc.tensor.matmul(
                            ps[:],
                            lhsT=aT_sb[kj][:, ts(t, 128)],
                            rhs=wo_sb[kj][:, ts(n, 512)],
                            start=(kj == 0),
                            stop=(kj == 1),
                        )
                    if (2 * t + n) % 2 == 0:
                        nc.scalar.copy(ysb[:, ts(n, 512)], ps[:])
                    else:
                        nc.vector.tensor_copy(ysb[:, ts(n, 512)], ps[:])
                nc.sync.dma_start(y_d[ts(t, 128), :], ysb[:])

    nc.compile()
    return nc


def make_masks():
    one = np.ones((128, 128), np.float32)
    maskd = np.triu(one)          # keep iff i >= kk  (diag tile)
    masks_ = np.tril(one, -1)     # keep iff i <  kk  (strict tile)
    md2 = np.concatenate([maskd, maskd], axis=1).astype(BF16)
    ms2 = np.concatenate([masks_, masks_], axis=1).astype(BF16)
    return md2, ms2


def make_in_maps(x, wq, wk, wv, wo):
    x = np.asarray(x, np.float32)
    wq, wk, wv, wo = (np.asarray(a, np.float32) for a in (wq, wk, wv, wo))
    md2, ms2 = make_masks()
    mask_all = np.hstack([md2, ms2])  # [128, 512]

    def tile_rows(a):  # [1024, W] -> [128, 8*W] (row-blocks side by side)
        return np.hstack([a[i * 128 : (i + 1) * 128] for i in range(a.shape[0] // 128)])

    in_maps = []
    for c in range(8):
        b, g = divmod(c, 4)
        sl = slice(g * CL, (g + 1) * CL)
        xTt = tile_rows(np.ascontiguousarray(x[b].T).astype(BF16))
        wt = np.hstack(
            [
                tile_rows(np.ascontiguousarray(w[sl, :].T).astype(BF16))
                for w in (wq, wk, wv)
            ]
        )
        wot = tile_rows(np.ascontiguousarray(wo[:, sl].T).astype(BF16))
        in_maps.append(
            {"xTt": xTt, "wt": wt, "wot": wot, "maskt": mask_all}
        )
    return in_maps


_PROG = None


def _get_prog():
    global _PROG
    if _PROG is None:
        _PROG = build_program()
    return _PROG


def kernel(x, wq, wk, wv, wo, _trace=False, _tmpdir=None):
    from concourse.bass_utils import run_bass_kernel_spmd

    nc = _get_prog()
    in_maps = make_in_maps(x, wq, wk, wv, wo)
    res = run_bass_kernel_spmd(
        nc, in_maps, core_ids=list(range(8)), trace=_trace, tmpdir=_tmpdir
    )
    y = np.zeros((B, T, C), np.float32)
    for c in range(8):
        b = c // 4
        y[b] += res.results[c]["y"].astype(np.float32)
    if _trace:
        kernel._last_results = res
    return y
